# revision 44
# baseline (speedup 1.0000x reference)
"""Linformer-style multihead attention on 8 Trainium2 NeuronCores.

Shapes (hardcoded): B=4, S=8192, D=512, H=8, DK=DV=64, PK=256.

Sharding: core c handles batch b=c//2, sequence half h=c%2 (4096 query rows).
The Linformer K/V projections contract over the full sequence: each core
computes the partial VPT/VFT = (value_half^T @ [We|Wf]) over its own 4096
rows, then a pair-wise HBM AllReduce completes the contraction while the
tensor engine runs the (independent) query projection.

Layout/pipeline notes:
- query arrives pre-transposed from the host (layout-only prep), so no
  DMA-transpose is needed (DMA transposes serialize against all other DMA
  traffic and stall the kernel start).
- Phase B computes the transposed projections directly (stationary operand =
  value d-block, moving = [We|Wf] concat), so VP^T/VF^T land feature-major in
  PSUM with no PE-transpose pass afterward.
- Phase D pipelines scores->exp->AV->Z-copy->recip->mult per (s-tile, head)
  slot with the previous s-tile's output projection interleaved as PE filler.
  The scalar engine issues the Z-copy of slot N before the exp of slot N+1 so
  the vector engine's normalization never blocks the next AV matmul.
- Output is written bf16 and upcast on host (halves output DMA).
- Biases fold in as rank-1 augmentation rows of the small matmuls:
  kh = VP@Wk + outer(sum(We,0), bk) + outer(1, be)   (and same for vh).
"""

import numpy as np
import ml_dtypes
from contextlib import ExitStack

import concourse.bass as bass
import concourse.bacc as bacc
import concourse.mybir as mybir
import concourse.tile as tile
from concourse import bass_utils

B, S, D = 4, 8192, 512
H, DK, DV, PK = 8, 64, 64, 256
SH = S // 2  # per-core query rows / value rows (phase-B split)
NCORES = 8
P = 128
PK2 = 2 * PK  # concat [We|Wf] output cols

F32 = mybir.dt.float32
BF16 = mybir.dt.bfloat16
AF = mybir.ActivationFunctionType
OP = mybir.AluOpType

_CACHE = {}


def _build_kernel(dbg=False):
    nc = bacc.Bacc(
        trn_type="TRN2",
        target_bir_lowering=False,
        debug=False,
        num_devices=NCORES,
    )

    qT_t = nc.dram_tensor("qT", [D, SH], BF16, kind="ExternalInput").ap()
    v_t = nc.dram_tensor("v", [SH, D], BF16, kind="ExternalInput").ap()
    wef_t = nc.dram_tensor("wef", [SH, PK2], BF16, kind="ExternalInput").ap()
    wq_t = nc.dram_tensor("wq", [D, D], BF16, kind="ExternalInput").ap()
    wk_t = nc.dram_tensor("wk", [D, D], BF16, kind="ExternalInput").ap()
    wv_t = nc.dram_tensor("wv", [D, D], BF16, kind="ExternalInput").ap()
    wo_t = nc.dram_tensor("wo", [D, D], BF16, kind="ExternalInput").ap()
    wkaug_t = nc.dram_tensor("wkaug", [2, D], BF16, kind="ExternalInput").ap()
    auge_t = nc.dram_tensor("auge", [2, PK], BF16, kind="ExternalInput").ap()
    wvaug_t = nc.dram_tensor("wvaug", [2, D], BF16, kind="ExternalInput").ap()
    augf_t = nc.dram_tensor("augf", [2, PK], BF16, kind="ExternalInput").ap()
    bq_t = nc.dram_tensor("bq", [D], F32, kind="ExternalInput").ap()
    bo_t = nc.dram_tensor("bo", [D], F32, kind="ExternalInput").ap()
    out_t = nc.dram_tensor("out", [SH, D], BF16, kind="ExternalOutput").ap()

    if dbg:
        dbg_vptf = nc.dram_tensor("dbg_vptf", [P, 4, PK2], BF16, kind="ExternalOutput").ap()
        dbg_khT = nc.dram_tensor("dbg_khT", [P, 4, PK], BF16, kind="ExternalOutput").ap()
        dbg_vh = nc.dram_tensor("dbg_vh", [P, 2, H, P], BF16, kind="ExternalOutput").ap()
        dbg_qhT = nc.dram_tensor("dbg_qhT", [P, 4, SH], BF16, kind="ExternalOutput").ap()
        dbg_e = nc.dram_tensor("dbg_e", [P, 2, 512], BF16, kind="ExternalOutput").ap()
        dbg_n = nc.dram_tensor("dbg_n", [P, 512], F32, kind="ExternalOutput").ap()
        dbg_rz = nc.dram_tensor("dbg_rz", [P, 512], F32, kind="ExternalOutput").ap()
        dbg_av = nc.dram_tensor("dbg_av", [P, 4, 512], BF16, kind="ExternalOutput").ap()

    NT = SH // 512  # 8 s-tiles of 512

    with ExitStack() as ctx:
        tc = ctx.enter_context(tile.TileContext(nc))
        consts = ctx.enter_context(tc.tile_pool(name="consts", bufs=1))
        big = ctx.enter_context(tc.tile_pool(name="big", bufs=1))
        dram = ctx.enter_context(tc.tile_pool(name="dram", bufs=1, space="DRAM"))

        # ---- persistent activations ----
        qTraw = big.tile([P, 4, SH], BF16)   # q transposed (feature-major)
        qhT = big.tile([P, 4, SH], BF16)     # q-proj, feature-major, scaled+bias
        vptf_p = big.tile([P, 4, PK2], BF16)  # partial VPT|VFT (this core's rows)
        vptf = big.tile([P, 4, PK2], BF16)    # reduced VPT|VFT
        khT = big.tile([P, 4, PK], BF16)     # [dk(2 heads/row-block), pair, pk]
        vh_sb = big.tile([P, 2, H, P], BF16)  # [pk rows, chunk, head, dv + 64 ones]

        # ---- small const DMAs (scalar HWDGE queue) ----
        wkaug_sb = consts.tile([2, D], BF16)
        nc.scalar.dma_start(out=wkaug_sb, in_=wkaug_t)
        auge_sb = consts.tile([2, PK], BF16)
        nc.scalar.dma_start(out=auge_sb, in_=auge_t)
        wvaug_sb = consts.tile([2, D], BF16)
        nc.scalar.dma_start(out=wvaug_sb, in_=wvaug_t)
        augf_sb = consts.tile([2, PK], BF16)
        nc.scalar.dma_start(out=augf_sb, in_=augf_t)
        bq_sb = consts.tile([P, 4], F32)
        nc.scalar.dma_start(out=bq_sb, in_=bq_t.rearrange("(c p) -> p c", p=P))
        bo_sb = consts.tile([P, D], F32)
        bo_bcast = bass.AP(tensor=bo_t.tensor, offset=bo_t.offset,
                           ap=[[0, P]] + list(bo_t.ap))
        nc.scalar.dma_start(out=bo_sb, in_=bo_bcast)

        wq_sb = consts.tile([P, 4, D], BF16)
        wk_sb = consts.tile([P, 4, D], BF16)
        wv_sb = consts.tile([P, 4, D], BF16)
        wo_sb = consts.tile([P, 4, D], BF16)

        nc.vector.memset(vh_sb[:, :, :, 64:], 1.0)
        warm_e = consts.tile([P, 4], F32)
        nc.scalar.activation(out=warm_e, in_=bq_sb, func=AF.Exp)

        # ---- phase B: partial VPT|VFT = value_half^T @ [We|Wf] ----
        # The big weight tensors and the pre-transposed query stream on the
        # same SWDGE queue BEHIND the phase-B chunks they must not starve:
        # the PE needs the value/[We|Wf] stream first, wq next (query
        # projection starts right after phase B), then the qT quarters just
        # ahead of the projection tiles that consume them.
        v_r = v_t.rearrange("(n p) d -> p n d", p=P)       # [128, 32, 512]
        wef_r = wef_t.rearrange("(n p) k -> p n k", p=P)   # [128, 32, 512]
        qT_r = qT_t.rearrange("(c p) s -> p c s", p=P)     # [128, 4, 4096]
        NCH = SH // P   # 32 chunks of 128 seq rows
        groups = [[0, 1], [2, 3], [4, 5], [6, 7]]
        sched = [2, 2, 4, 8, 8, 8]
        assert sum(sched) == NCH
        late_loads = [
            lambda: nc.gpsimd.dma_start(
                out=wq_sb, in_=wq_t.rearrange("(c p) e -> p c e", p=P)),
            lambda: nc.gpsimd.dma_start(
                out=qTraw[:, :, 0:1024], in_=qT_r[:, :, 0:1024]),
            lambda: nc.gpsimd.dma_start(
                out=qTraw[:, :, 1024:2048], in_=qT_r[:, :, 1024:2048]),
            lambda: nc.gpsimd.dma_start(
                out=qTraw[:, :, 2048:3072], in_=qT_r[:, :, 2048:3072]),
            lambda: nc.gpsimd.dma_start(
                out=qTraw[:, :, 3072:4096], in_=qT_r[:, :, 3072:4096]),
            lambda: nc.gpsimd.dma_start(
                out=wk_sb, in_=wk_t.rearrange("(c p) e -> p c e", p=P)),
            lambda: nc.gpsimd.dma_start(
                out=wv_sb, in_=wv_t.rearrange("(c p) e -> p c e", p=P)),
            lambda: nc.gpsimd.dma_start(
                out=wo_sb, in_=wo_t.rearrange("(c p) e -> p c e", p=P)),
        ]
        with (
            tc.tile_pool(name="vstream", bufs=5) as vstream,
            tc.tile_pool(name="wstream", bufs=5) as wstream,
            tc.tile_pool(name="accp", bufs=4, space="PSUM") as accp,
        ):
            ps_dc = [accp.tile([P, PK2], F32, tag="acc", name=f"bps{i}")
                     for i in range(4)]
            k0 = 0
            for sci, ch in enumerate(sched):
                val_sb = vstream.tile([P, 8, D], BF16, tag="val")
                nc.gpsimd.dma_start(out=val_sb[:, 0:ch, :],
                                    in_=v_r[:, k0:k0 + ch, :])
                wef_sb = wstream.tile([P, 8, PK2], BF16, tag="wef")
                nc.gpsimd.dma_start(out=wef_sb[:, 0:ch, :],
                                    in_=wef_r[:, k0:k0 + ch, :])
                if sci == 0:
                    # tiny warmup collective: pays the TOPSP firmware wakeup
                    # and pair-sync while the PE streams phase B, so the real
                    # AllReduce below begins ~immediately. Sits behind the
                    # first stream DMAs so its trigger overhead cannot delay
                    # the PE start.
                    w_ci = dram.tile([2, 16], BF16, name="warm_ci")
                    w_co = dram.tile([2, 16], BF16, name="warm_co")
                    nc.gpsimd.collective_compute(
                        "AllReduce", OP.add, replica_groups=groups,
                        ins=[w_ci.opt()], outs=[w_co.opt()])
                for i in range(ch):
                    k = k0 + i
                    first, last = (k == 0), (k == NCH - 1)
                    for dc in range(4):
                        nc.tensor.matmul(
                            ps_dc[dc], lhsT=val_sb[:, i, dc * P:(dc + 1) * P],
                            rhs=wef_sb[:, i, :], start=first, stop=last)
                k0 += ch
            # FIFO order on the SWDGE queue after the B stream: wq + first two
            # qT quarters (query projection starts right after phase B), then
            # the collective (its input DMA only waits on the PSUM->SBUF
            # copies), then the remaining loads, which are needed later.
            for ld in late_loads[0:3]:
                ld()
            for dc in range(4):
                nc.scalar.activation(out=vptf_p[:, dc, :], in_=ps_dc[dc],
                                     func=AF.Copy)
            ci = dram.tile([P, 4 * PK2], BF16, name="cc_in")
            co = dram.tile([P, 4 * PK2], BF16, name="cc_out")
            nc.gpsimd.dma_start(out=ci, in_=vptf_p.rearrange("p a b -> p (a b)"))
            nc.gpsimd.collective_compute(
                "AllReduce", OP.add, replica_groups=groups,
                ins=[ci.opt()], outs=[co.opt()])
            for ld in late_loads[3:]:
                ld()

        # ---- phase C: qhT = (Wq^T @ queryT) + bq (independent of AllReduce,
        # keeps the PE busy while the collective completes) ----
        with tc.tile_pool(name="qp", bufs=4, space="PSUM") as qp:
            for sg in range(4):
                ssl = slice(sg * 1024, (sg + 1) * 1024)
                for eb in range(4):
                    ps_t = qp.tile([P, 2, 512], F32, tag="q")
                    for dc in range(4):
                        for hf in range(2):
                            nc.tensor.matmul(
                                ps_t[:, hf, :],
                                lhsT=wq_sb[:, dc, eb * P:(eb + 1) * P],
                                rhs=qTraw[:, dc,
                                          sg * 1024 + hf * 512:
                                          sg * 1024 + (hf + 1) * 512],
                                start=(dc == 0), stop=(dc == 3))
                    nc.vector.tensor_scalar(
                        out=qhT[:, eb, ssl].rearrange("p (a b) -> p a b", a=2),
                        in0=ps_t, scalar1=bq_sb[:, eb:eb + 1], scalar2=None,
                        op0=OP.add)

        # read back the reduced projections (the gpsimd DMA waits on the
        # collective without blocking the PE's query projection above)
        nc.gpsimd.dma_start(out=vptf.rearrange("p a b -> p (a b)"), in_=co)

        # ---- khT[e, pk] = Wk^T @ VPT + rank-1 bias rows ----
        with tc.tile_pool(name="khp", bufs=2, space="PSUM") as khp:
            for pr in range(4):
                ps_t = khp.tile([P, PK], F32, tag="kh")
                for dc in range(4):
                    nc.tensor.matmul(
                        ps_t, lhsT=wk_sb[:, dc, pr * P:(pr + 1) * P],
                        rhs=vptf[:, dc, 0:PK], start=(dc == 0), stop=False)
                nc.tensor.matmul(
                    ps_t, lhsT=wkaug_sb[:, pr * P:(pr + 1) * P],
                    rhs=auge_sb, start=False, stop=True)
                nc.vector.tensor_copy(out=khT[:, pr, :], in_=ps_t)

        # ---- vh[pk, dv] = VFT^T @ Wv + rank-1 bias rows (seq-major in pk) ----
        with tc.tile_pool(name="vhp", bufs=2, space="PSUM") as vhp:
            for ps in range(2):
                ps_t = vhp.tile([P, D], F32, tag="vh")
                for dc in range(4):
                    nc.tensor.matmul(
                        ps_t, lhsT=vptf[:, dc, PK + ps * P:PK + (ps + 1) * P],
                        rhs=wv_sb[:, dc, :], start=(dc == 0), stop=False)
                nc.tensor.matmul(
                    ps_t, lhsT=augf_sb[:, ps * P:(ps + 1) * P],
                    rhs=wvaug_sb, start=False, stop=True)
                nc.vector.tensor_copy(
                    out=vh_sb[:, ps, :, 0:64],
                    in_=ps_t.rearrange("p (h v) -> p h v", h=H))

        if dbg:
            nc.gpsimd.dma_start(out=dbg_vptf, in_=vptf)
            nc.gpsimd.dma_start(out=dbg_khT, in_=khT)
            nc.gpsimd.dma_start(out=dbg_vh, in_=vh_sb)
            nc.gpsimd.dma_start(out=dbg_qhT, in_=qhT)

        # ---- phase D: attention + output projection, software-pipelined ----
        out_r = out_t.rearrange("(t c p) d -> t p c d", c=4, p=P)
        with (
            tc.tile_pool(name="scp", bufs=2, space="PSUM") as scp,
            tc.tile_pool(name="nump", bufs=2, space="PSUM") as nump,
            tc.tile_pool(name="outp", bufs=2, space="PSUM") as outp,
            tc.tile_pool(name="epool", bufs=4) as epool,
            tc.tile_pool(name="rzp", bufs=12) as rzp,
            tc.tile_pool(name="avp", bufs=3) as avp,
            tc.tile_pool(name="ostage", bufs=3) as ostage,
        ):
            sc_tiles = {}
            e_tiles = {}
            av_tiles = {}
            o_state = {}

            def issue_scores_mm(st, h):
                pr, hb = h // 2, (h % 2) * 64
                ssl = slice(st * 512, (st + 1) * 512)
                sc_t = scp.tile([P, 2, 512], F32, tag="sc", name=f"sc{st}_{h}")
                for ps in range(2):
                    nc.tensor.matmul(
                        sc_t[:, ps, :],
                        lhsT=khT[hb:hb + 64, pr, ps * P:(ps + 1) * P],
                        rhs=qhT[hb:hb + 64, pr, ssl],
                        start=True, stop=True)
                sc_tiles[(st, h)] = sc_t

            def issue_exp(st, h):
                sc_t = sc_tiles.pop((st, h))
                e_t = epool.tile([P, 2, 512], BF16, tag="e", name=f"e{st}_{h}")
                nc.scalar.activation(out=e_t, in_=sc_t, func=AF.Exp)
                if dbg and st == 0 and h == 0:
                    nc.gpsimd.dma_start(out=dbg_e, in_=e_t)
                e_tiles[(st, h)] = e_t

            def issue_av(st, h):
                pr, hb = h // 2, (h % 2) * 64
                e_t = e_tiles.pop((st, h))
                if h == 0:
                    av_tiles[st] = avp.tile([P, 4, 512], BF16, tag="av",
                                            name=f"av{st}")
                n_t = nump.tile([P, 512], F32, tag="num")
                for c in range(2):
                    nc.tensor.matmul(
                        n_t, lhsT=vh_sb[:, c, h, :], rhs=e_t[:, c, :],
                        start=(c == 0), stop=(c == 1))
                zz = rzp.tile([64, 512], F32, tag="zz", name=f"zz{st}_{h}")
                if h == H - 1:
                    nc.vector.tensor_copy(out=zz, in_=n_t[64:P, :])
                else:
                    nc.scalar.activation(out=zz, in_=n_t[64:P, :], func=AF.Copy)
                rzb = rzp.tile([64, 512], F32, tag="rzb", name=f"rzb{st}_{h}")
                nc.vector.reciprocal_approx_fast(out=rzb, in_=zz)
                if dbg and st == 0 and h == 0:
                    n_cp = rzp.tile([P, 512], F32, tag="ncp", name="ncp")
                    nc.vector.tensor_copy(out=n_cp, in_=n_t)
                    nc.gpsimd.dma_start(out=dbg_n, in_=n_cp)
                    nc.gpsimd.dma_start(out=dbg_rz[64:P, :], in_=rzb)
                nc.vector.tensor_tensor(
                    out=av_tiles[st][hb:hb + 64, pr, :],
                    in0=n_t[0:64, :], in1=rzb, op=OP.mult)

            def outproj_pieces(st):
                """Yield callables: 16 accum-matmuls + bias-adds + out DMA."""
                av_t = av_tiles[st]
                o_sb = ostage.tile([P, 4, D], BF16, tag="ost", name=f"ost{st}")
                o_state[st] = o_sb

                def mk_mm(sl, pr):
                    def go():
                        if pr == 0:
                            o_state[(st, sl)] = outp.tile(
                                [P, D], F32, tag="o", name=f"o{st}_{sl}")
                        nc.tensor.matmul(
                            o_state[(st, sl)],
                            lhsT=av_t[:, pr, sl * P:(sl + 1) * P],
                            rhs=wo_sb[:, pr, :], start=(pr == 0), stop=(pr == 3))
                        if pr == 3:
                            nc.vector.tensor_tensor(
                                out=o_sb[:, sl, :], in0=o_state.pop((st, sl)),
                                in1=bo_sb, op=OP.add)
                    return go

                for sl in range(4):
                    for pr in range(4):
                        yield mk_mm(sl, pr)

                def mk_dma(sl):
                    def go():
                        if dbg and st == 0 and sl == 0:
                            nc.gpsimd.dma_start(out=dbg_av, in_=av_t)
                        if sl == 0:
                            av_tiles.pop(st)
                        nc.sync.dma_start(out=out_r[st][:, sl, :],
                                          in_=o_state[st][:, sl, :])
                        if sl == 3:
                            o_state.pop(st)
                    return go

                for sl in range(4):
                    yield mk_dma(sl)

            # slot loop: scores MMs one slot ahead (PE), Z-copy of the current
            # slot goes to the scalar engine BEFORE the next slot's exp, and
            # the previous s-tile's output projection fills PE gaps.
            slots = [(st, h) for st in range(NT) for h in range(H)]
            filler = None
            issue_scores_mm(0, 0)
            issue_exp(0, 0)
            for idx, (st, h) in enumerate(slots):
                if idx + 1 < len(slots):
                    issue_scores_mm(*slots[idx + 1])
                    issue_exp(*slots[idx + 1])
                issue_av(st, h)
                if h == H - 1:
                    if filler is not None:
                        for f in filler:
                            f()
                    filler = outproj_pieces(st)
                    if st == NT - 1:
                        for f in filler:
                            f()
                        filler = None
                elif filler is not None and h >= 1:
                    for _ in range(3):
                        f = next(filler, None)
                        if f is None:
                            filler = None
                            break
                        f()
            assert filler is None

    nc.finalize()
    return nc


def _prep_inputs(inputs):
    bf = ml_dtypes.bfloat16
    f32 = np.float32
    q = np.ascontiguousarray(inputs["query"])
    v = np.ascontiguousarray(inputs["value"])
    We, Wf = np.asarray(inputs["We"]), np.asarray(inputs["Wf"])
    scale = np.float32(DK ** -0.5)
    ones = np.ones(D, f32)
    sWe = We.astype(f32).sum(0)
    sWf = Wf.astype(f32).sum(0)
    wef = np.concatenate([We, Wf], axis=1).astype(bf)
    shared = {
        "wq": (np.asarray(inputs["Wq"]) * scale).astype(bf),
        "wk": np.asarray(inputs["Wk"]).astype(bf),
        "wv": np.asarray(inputs["Wv"]).astype(bf),
        "wo": np.asarray(inputs["Wo"]).astype(bf),
        "wkaug": np.stack([np.asarray(inputs["bk"], f32), ones]).astype(bf),
        "auge": np.stack([sWe, np.asarray(inputs["be"], f32)]).astype(bf),
        "wvaug": np.stack([np.asarray(inputs["bv"], f32), ones]).astype(bf),
        "augf": np.stack([sWf, np.asarray(inputs["bf"], f32)]).astype(bf),
        "bq": (np.asarray(inputs["bq"]) * scale).astype(f32),
        "bo": np.asarray(inputs["bo"]).astype(f32),
    }
    in_maps = []
    for c in range(NCORES):
        b, half = c // 2, c % 2
        sl = slice(half * SH, (half + 1) * SH)
        m = dict(shared)
        m["qT"] = np.ascontiguousarray(q[b, sl, :].T).astype(bf)
        m["v"] = np.ascontiguousarray(v[b, sl, :]).astype(bf)
        m["wef"] = np.ascontiguousarray(wef[sl, :])
        in_maps.append(m)
    return in_maps


def kernel(**inputs):
    if "nc" not in _CACHE:
        _CACHE["nc"] = _build_kernel()
    nc = _CACHE["nc"]
    in_maps = _prep_inputs(inputs)
    res = bass_utils.run_bass_kernel_spmd(nc, in_maps, core_ids=list(range(NCORES)))
    out = np.empty((B, S, D), np.float32)
    for c in range(NCORES):
        b, half = c // 2, c % 2
        out[b, half * SH:(half + 1) * SH, :] = res.results[c]["out"].astype(np.float32)
    return out


# revision 45
# speedup vs baseline: 1.1621x; 1.1621x over previous
"""Linformer-style multihead attention on 8 Trainium2 NeuronCores.

Shapes (hardcoded): B=4, S=8192, D=512, H=8, DK=DV=64, PK=256.

Sharding: core c handles batch b=c//2, sequence half h=c%2 (4096 query rows).
The Linformer K/V projections contract over the full sequence: each core
computes the partial VPT/VFT = (value_half^T @ [We|Wf]) over its own 4096
rows, then a pair-wise HBM AllReduce completes the contraction while the
tensor engine runs the (independent) query projection.

Layout/pipeline notes:
- query arrives pre-transposed from the host (layout-only prep), so no
  DMA-transpose is needed (DMA transposes serialize against all other DMA
  traffic and stall the kernel start).
- Phase B computes the transposed projections directly (stationary operand =
  value d-block, moving = [We|Wf] concat), so VP^T/VF^T land feature-major in
  PSUM with no PE-transpose pass afterward.
- Phase D pipelines scores->exp->AV->Z-copy->recip->mult per (s-tile, head)
  slot with the previous s-tile's output projection interleaved as PE filler.
  The scalar engine issues the Z-copy of slot N before the exp of slot N+1 so
  the vector engine's normalization never blocks the next AV matmul.
- Output is written bf16 and upcast on host (halves output DMA).
- Biases fold in as rank-1 augmentation rows of the small matmuls:
  kh = VP@Wk + outer(sum(We,0), bk) + outer(1, be)   (and same for vh).
"""

import numpy as np
import ml_dtypes
from contextlib import ExitStack

import concourse.bass as bass
import concourse.bacc as bacc
import concourse.mybir as mybir
import concourse.tile as tile
from concourse import bass_utils

B, S, D = 4, 8192, 512
H, DK, DV, PK = 8, 64, 64, 256
SH = S // 2  # per-core query rows / value rows (phase-B split)
NCORES = 8
P = 128
PK2 = 2 * PK  # concat [We|Wf] output cols

F32 = mybir.dt.float32
BF16 = mybir.dt.bfloat16
AF = mybir.ActivationFunctionType
OP = mybir.AluOpType

_CACHE = {}


def _build_kernel(dbg=False):
    nc = bacc.Bacc(
        trn_type="TRN2",
        target_bir_lowering=False,
        debug=False,
        num_devices=NCORES,
    )

    qT_t = nc.dram_tensor("qT", [D, SH], BF16, kind="ExternalInput").ap()
    v_t = nc.dram_tensor("v", [SH, D], BF16, kind="ExternalInput").ap()
    wef_t = nc.dram_tensor("wef", [SH, PK2], BF16, kind="ExternalInput").ap()
    wq_t = nc.dram_tensor("wq", [D, D], BF16, kind="ExternalInput").ap()
    wk_t = nc.dram_tensor("wk", [D, D], BF16, kind="ExternalInput").ap()
    wv_t = nc.dram_tensor("wv", [D, D], BF16, kind="ExternalInput").ap()
    wo_t = nc.dram_tensor("wo", [D, D], BF16, kind="ExternalInput").ap()
    wkaug_t = nc.dram_tensor("wkaug", [2, D], BF16, kind="ExternalInput").ap()
    auge_t = nc.dram_tensor("auge", [2, PK], BF16, kind="ExternalInput").ap()
    wvaug_t = nc.dram_tensor("wvaug", [2, D], BF16, kind="ExternalInput").ap()
    augf_t = nc.dram_tensor("augf", [2, PK], BF16, kind="ExternalInput").ap()
    bq_t = nc.dram_tensor("bq", [D], F32, kind="ExternalInput").ap()
    bo_t = nc.dram_tensor("bo", [D], F32, kind="ExternalInput").ap()
    out_t = nc.dram_tensor("out", [SH, D], BF16, kind="ExternalOutput").ap()

    if dbg:
        dbg_vptf = nc.dram_tensor("dbg_vptf", [P, 4, PK2], BF16, kind="ExternalOutput").ap()
        dbg_khT = nc.dram_tensor("dbg_khT", [P, 4, PK], BF16, kind="ExternalOutput").ap()
        dbg_vh = nc.dram_tensor("dbg_vh", [P, 2, H, P], BF16, kind="ExternalOutput").ap()
        dbg_qhT = nc.dram_tensor("dbg_qhT", [P, 4, SH], BF16, kind="ExternalOutput").ap()
        dbg_e = nc.dram_tensor("dbg_e", [P, 2, 512], BF16, kind="ExternalOutput").ap()
        dbg_n = nc.dram_tensor("dbg_n", [P, 512], F32, kind="ExternalOutput").ap()
        dbg_rz = nc.dram_tensor("dbg_rz", [P, 512], F32, kind="ExternalOutput").ap()
        dbg_av = nc.dram_tensor("dbg_av", [P, 4, 512], BF16, kind="ExternalOutput").ap()

    NT = SH // 512  # 8 s-tiles of 512

    with ExitStack() as ctx:
        tc = ctx.enter_context(tile.TileContext(nc))
        consts = ctx.enter_context(tc.tile_pool(name="consts", bufs=1))
        big = ctx.enter_context(tc.tile_pool(name="big", bufs=1))
        dram = ctx.enter_context(tc.tile_pool(name="dram", bufs=1, space="DRAM"))

        # ---- persistent activations ----
        qTraw = big.tile([P, 4, SH], BF16)   # q transposed (feature-major)
        qhT = big.tile([P, 4, SH], BF16)     # q-proj, feature-major, scaled+bias
        vptf_p = big.tile([P, 4, PK2], BF16)  # partial VPT|VFT (this core's rows)
        vptf = big.tile([P, 4, PK2], BF16)    # reduced VPT|VFT
        khT = big.tile([P, 4, PK], BF16)     # [dk(2 heads/row-block), pair, pk]
        vh_sb = big.tile([P, 2, H, P], BF16)  # [pk rows, chunk, head, dv + 64 ones]

        # ---- small const DMAs (scalar HWDGE queue) ----
        wkaug_sb = consts.tile([2, D], BF16)
        nc.scalar.dma_start(out=wkaug_sb, in_=wkaug_t)
        auge_sb = consts.tile([2, PK], BF16)
        nc.scalar.dma_start(out=auge_sb, in_=auge_t)
        wvaug_sb = consts.tile([2, D], BF16)
        nc.scalar.dma_start(out=wvaug_sb, in_=wvaug_t)
        augf_sb = consts.tile([2, PK], BF16)
        nc.scalar.dma_start(out=augf_sb, in_=augf_t)
        bq_sb = consts.tile([P, 4], F32)
        nc.scalar.dma_start(out=bq_sb, in_=bq_t.rearrange("(c p) -> p c", p=P))
        bo_sb = consts.tile([P, D], F32)
        bo_bcast = bass.AP(tensor=bo_t.tensor, offset=bo_t.offset,
                           ap=[[0, P]] + list(bo_t.ap))
        nc.scalar.dma_start(out=bo_sb, in_=bo_bcast)

        wq_sb = consts.tile([P, 4, D], BF16)
        wk_sb = consts.tile([P, 4, D], BF16)
        wv_sb = consts.tile([P, 4, D], BF16)
        wo_sb = consts.tile([P, 4, D], BF16)

        nc.vector.memset(vh_sb[:, :, :, 64:], 1.0)
        warm_e = consts.tile([P, 4], F32)
        nc.scalar.activation(out=warm_e, in_=bq_sb, func=AF.Exp)

        # ---- phase B: partial VPT|VFT = value_half^T @ [We|Wf] ----
        # The big weight tensors and the pre-transposed query stream on the
        # same SWDGE queue BEHIND the phase-B chunks they must not starve:
        # the PE needs the value/[We|Wf] stream first, wq next (query
        # projection starts right after phase B), then the qT quarters just
        # ahead of the projection tiles that consume them.
        v_r = v_t.rearrange("(n p) d -> p n d", p=P)       # [128, 32, 512]
        wef_r = wef_t.rearrange("(n p) k -> p n k", p=P)   # [128, 32, 512]
        qT_r = qT_t.rearrange("(c p) s -> p c s", p=P)     # [128, 4, 4096]
        NCH = SH // P   # 32 chunks of 128 seq rows
        groups = [[0, 1], [2, 3], [4, 5], [6, 7]]
        sched = [2, 2, 4, 8, 8, 8]
        assert sum(sched) == NCH
        # tiny warmup collective at kernel entry: pays the TOPSP firmware
        # wakeup latency and pair-syncs the cores while the PE streams
        # phase B, so the real AllReduce below begins ~immediately
        w_ci = dram.tile([2, 16], BF16, name="warm_ci")
        w_co = dram.tile([2, 16], BF16, name="warm_co")
        nc.gpsimd.collective_compute(
            "AllReduce", OP.add, replica_groups=groups,
            ins=[w_ci.opt()], outs=[w_co.opt()])
        late_loads = [
            lambda: nc.gpsimd.dma_start(
                out=wq_sb, in_=wq_t.rearrange("(c p) e -> p c e", p=P)),
            lambda: nc.gpsimd.dma_start(
                out=qTraw[:, :, 0:1024], in_=qT_r[:, :, 0:1024]),
            lambda: nc.gpsimd.dma_start(
                out=qTraw[:, :, 1024:2048], in_=qT_r[:, :, 1024:2048]),
            lambda: nc.gpsimd.dma_start(
                out=qTraw[:, :, 2048:3072], in_=qT_r[:, :, 2048:3072]),
            lambda: nc.gpsimd.dma_start(
                out=qTraw[:, :, 3072:4096], in_=qT_r[:, :, 3072:4096]),
            lambda: nc.gpsimd.dma_start(
                out=wk_sb, in_=wk_t.rearrange("(c p) e -> p c e", p=P)),
            lambda: nc.gpsimd.dma_start(
                out=wv_sb, in_=wv_t.rearrange("(c p) e -> p c e", p=P)),
            lambda: nc.gpsimd.dma_start(
                out=wo_sb, in_=wo_t.rearrange("(c p) e -> p c e", p=P)),
        ]
        with (
            tc.tile_pool(name="vstream", bufs=5) as vstream,
            tc.tile_pool(name="wstream", bufs=5) as wstream,
            tc.tile_pool(name="accp", bufs=4, space="PSUM") as accp,
        ):
            ps_dc = [accp.tile([P, PK2], F32, tag="acc", name=f"bps{i}")
                     for i in range(4)]
            k0 = 0
            for sci, ch in enumerate(sched):
                val_sb = vstream.tile([P, 8, D], BF16, tag="val")
                nc.gpsimd.dma_start(out=val_sb[:, 0:ch, :],
                                    in_=v_r[:, k0:k0 + ch, :])
                wef_sb = wstream.tile([P, 8, PK2], BF16, tag="wef")
                nc.gpsimd.dma_start(out=wef_sb[:, 0:ch, :],
                                    in_=wef_r[:, k0:k0 + ch, :])
                for i in range(ch):
                    k = k0 + i
                    first, last = (k == 0), (k == NCH - 1)
                    for dc in range(4):
                        nc.tensor.matmul(
                            ps_dc[dc], lhsT=val_sb[:, i, dc * P:(dc + 1) * P],
                            rhs=wef_sb[:, i, :], start=first, stop=last)
                k0 += ch
            # FIFO order on the SWDGE queue after the B stream: wq + first two
            # qT quarters (query projection starts right after phase B), then
            # the collective (its input DMA only waits on the PSUM->SBUF
            # copies), then the remaining loads, which are needed later.
            for ld in late_loads[0:3]:
                ld()
            for dc in range(4):
                nc.scalar.activation(out=vptf_p[:, dc, :], in_=ps_dc[dc],
                                     func=AF.Copy)
            ci = dram.tile([P, 4 * PK2], BF16, name="cc_in")
            co = dram.tile([P, 4 * PK2], BF16, name="cc_out")
            nc.gpsimd.dma_start(out=ci, in_=vptf_p.rearrange("p a b -> p (a b)"))
            nc.gpsimd.collective_compute(
                "AllReduce", OP.add, replica_groups=groups,
                ins=[ci.opt()], outs=[co.opt()])
            for ld in late_loads[3:]:
                ld()

        # ---- phase C: qhT = (Wq^T @ queryT) + bq (independent of AllReduce,
        # keeps the PE busy while the collective completes) ----
        with tc.tile_pool(name="qp", bufs=4, space="PSUM") as qp:
            for sg in range(4):
                ssl = slice(sg * 1024, (sg + 1) * 1024)
                for eb in range(4):
                    ps_t = qp.tile([P, 2, 512], F32, tag="q")
                    for dc in range(4):
                        for hf in range(2):
                            nc.tensor.matmul(
                                ps_t[:, hf, :],
                                lhsT=wq_sb[:, dc, eb * P:(eb + 1) * P],
                                rhs=qTraw[:, dc,
                                          sg * 1024 + hf * 512:
                                          sg * 1024 + (hf + 1) * 512],
                                start=(dc == 0), stop=(dc == 3))
                    nc.vector.tensor_scalar(
                        out=qhT[:, eb, ssl].rearrange("p (a b) -> p a b", a=2),
                        in0=ps_t, scalar1=bq_sb[:, eb:eb + 1], scalar2=None,
                        op0=OP.add)

        # read back the reduced projections (the gpsimd DMA waits on the
        # collective without blocking the PE's query projection above)
        nc.gpsimd.dma_start(out=vptf.rearrange("p a b -> p (a b)"), in_=co)

        # ---- khT[e, pk] = Wk^T @ VPT + rank-1 bias rows ----
        with tc.tile_pool(name="khp", bufs=2, space="PSUM") as khp:
            for pr in range(4):
                ps_t = khp.tile([P, PK], F32, tag="kh")
                for dc in range(4):
                    nc.tensor.matmul(
                        ps_t, lhsT=wk_sb[:, dc, pr * P:(pr + 1) * P],
                        rhs=vptf[:, dc, 0:PK], start=(dc == 0), stop=False)
                nc.tensor.matmul(
                    ps_t, lhsT=wkaug_sb[:, pr * P:(pr + 1) * P],
                    rhs=auge_sb, start=False, stop=True)
                nc.vector.tensor_copy(out=khT[:, pr, :], in_=ps_t)

        # ---- vh[pk, dv] = VFT^T @ Wv + rank-1 bias rows (seq-major in pk) ----
        with tc.tile_pool(name="vhp", bufs=2, space="PSUM") as vhp:
            for ps in range(2):
                ps_t = vhp.tile([P, D], F32, tag="vh")
                for dc in range(4):
                    nc.tensor.matmul(
                        ps_t, lhsT=vptf[:, dc, PK + ps * P:PK + (ps + 1) * P],
                        rhs=wv_sb[:, dc, :], start=(dc == 0), stop=False)
                nc.tensor.matmul(
                    ps_t, lhsT=augf_sb[:, ps * P:(ps + 1) * P],
                    rhs=wvaug_sb, start=False, stop=True)
                nc.vector.tensor_copy(
                    out=vh_sb[:, ps, :, 0:64],
                    in_=ps_t.rearrange("p (h v) -> p h v", h=H))

        if dbg:
            nc.gpsimd.dma_start(out=dbg_vptf, in_=vptf)
            nc.gpsimd.dma_start(out=dbg_khT, in_=khT)
            nc.gpsimd.dma_start(out=dbg_vh, in_=vh_sb)
            nc.gpsimd.dma_start(out=dbg_qhT, in_=qhT)

        # ---- phase D: attention + output projection, software-pipelined ----
        out_r = out_t.rearrange("(t c p) d -> t p c d", c=4, p=P)
        with (
            tc.tile_pool(name="scp", bufs=2, space="PSUM") as scp,
            tc.tile_pool(name="nump", bufs=2, space="PSUM") as nump,
            tc.tile_pool(name="outp", bufs=2, space="PSUM") as outp,
            tc.tile_pool(name="epool", bufs=4) as epool,
            tc.tile_pool(name="rzp", bufs=12) as rzp,
            tc.tile_pool(name="avp", bufs=3) as avp,
            tc.tile_pool(name="ostage", bufs=3) as ostage,
        ):
            sc_tiles = {}
            e_tiles = {}
            av_tiles = {}
            o_state = {}

            def issue_scores_mm(st, h):
                pr, hb = h // 2, (h % 2) * 64
                ssl = slice(st * 512, (st + 1) * 512)
                sc_t = scp.tile([P, 2, 512], F32, tag="sc", name=f"sc{st}_{h}")
                for ps in range(2):
                    nc.tensor.matmul(
                        sc_t[:, ps, :],
                        lhsT=khT[hb:hb + 64, pr, ps * P:(ps + 1) * P],
                        rhs=qhT[hb:hb + 64, pr, ssl],
                        start=True, stop=True)
                sc_tiles[(st, h)] = sc_t

            def issue_exp(st, h):
                sc_t = sc_tiles.pop((st, h))
                e_t = epool.tile([P, 2, 512], BF16, tag="e", name=f"e{st}_{h}")
                nc.scalar.activation(out=e_t, in_=sc_t, func=AF.Exp)
                if dbg and st == 0 and h == 0:
                    nc.gpsimd.dma_start(out=dbg_e, in_=e_t)
                e_tiles[(st, h)] = e_t

            def issue_av(st, h):
                pr, hb = h // 2, (h % 2) * 64
                e_t = e_tiles.pop((st, h))
                if h == 0:
                    av_tiles[st] = avp.tile([P, 4, 512], BF16, tag="av",
                                            name=f"av{st}")
                n_t = nump.tile([P, 512], F32, tag="num")
                for c in range(2):
                    nc.tensor.matmul(
                        n_t, lhsT=vh_sb[:, c, h, :], rhs=e_t[:, c, :],
                        start=(c == 0), stop=(c == 1))
                zz = rzp.tile([64, 512], F32, tag="zz", name=f"zz{st}_{h}")
                if h == H - 1:
                    nc.vector.tensor_copy(out=zz, in_=n_t[64:P, :])
                else:
                    nc.scalar.activation(out=zz, in_=n_t[64:P, :], func=AF.Copy)
                rzb = rzp.tile([64, 512], F32, tag="rzb", name=f"rzb{st}_{h}")
                nc.vector.reciprocal_approx_fast(out=rzb, in_=zz)
                if dbg and st == 0 and h == 0:
                    n_cp = rzp.tile([P, 512], F32, tag="ncp", name="ncp")
                    nc.vector.tensor_copy(out=n_cp, in_=n_t)
                    nc.gpsimd.dma_start(out=dbg_n, in_=n_cp)
                    nc.gpsimd.dma_start(out=dbg_rz[64:P, :], in_=rzb)
                nc.vector.tensor_tensor(
                    out=av_tiles[st][hb:hb + 64, pr, :],
                    in0=n_t[0:64, :], in1=rzb, op=OP.mult)

            def outproj_pieces(st):
                """Yield callables: 16 accum-matmuls + bias-adds + out DMA."""
                av_t = av_tiles[st]
                o_sb = ostage.tile([P, 4, D], BF16, tag="ost", name=f"ost{st}")
                o_state[st] = o_sb

                def mk_mm(sl, pr):
                    def go():
                        if pr == 0:
                            o_state[(st, sl)] = outp.tile(
                                [P, D], F32, tag="o", name=f"o{st}_{sl}")
                        nc.tensor.matmul(
                            o_state[(st, sl)],
                            lhsT=av_t[:, pr, sl * P:(sl + 1) * P],
                            rhs=wo_sb[:, pr, :], start=(pr == 0), stop=(pr == 3))
                        if pr == 3:
                            nc.vector.tensor_tensor(
                                out=o_sb[:, sl, :], in0=o_state.pop((st, sl)),
                                in1=bo_sb, op=OP.add)
                    return go

                for sl in range(4):
                    for pr in range(4):
                        yield mk_mm(sl, pr)

                def mk_dma(sl):
                    def go():
                        if dbg and st == 0 and sl == 0:
                            nc.gpsimd.dma_start(out=dbg_av, in_=av_t)
                        if sl == 0:
                            av_tiles.pop(st)
                        nc.sync.dma_start(out=out_r[st][:, sl, :],
                                          in_=o_state[st][:, sl, :])
                        if sl == 3:
                            o_state.pop(st)
                    return go

                for sl in range(4):
                    yield mk_dma(sl)

            # slot loop: scores MMs one slot ahead (PE), Z-copy of the current
            # slot goes to the scalar engine BEFORE the next slot's exp, and
            # the previous s-tile's output projection fills PE gaps.
            slots = [(st, h) for st in range(NT) for h in range(H)]
            filler = None
            issue_scores_mm(0, 0)
            issue_exp(0, 0)
            for idx, (st, h) in enumerate(slots):
                if idx + 1 < len(slots):
                    issue_scores_mm(*slots[idx + 1])
                    issue_exp(*slots[idx + 1])
                issue_av(st, h)
                if h == H - 1:
                    if filler is not None:
                        for f in filler:
                            f()
                    filler = outproj_pieces(st)
                    if st == NT - 1:
                        for f in filler:
                            f()
                        filler = None
                elif filler is not None and h >= 1:
                    for _ in range(3):
                        f = next(filler, None)
                        if f is None:
                            filler = None
                            break
                        f()
            assert filler is None

    nc.finalize()
    return nc


def _prep_inputs(inputs):
    bf = ml_dtypes.bfloat16
    f32 = np.float32
    q = np.ascontiguousarray(inputs["query"])
    v = np.ascontiguousarray(inputs["value"])
    We, Wf = np.asarray(inputs["We"]), np.asarray(inputs["Wf"])
    scale = np.float32(DK ** -0.5)
    ones = np.ones(D, f32)
    sWe = We.astype(f32).sum(0)
    sWf = Wf.astype(f32).sum(0)
    wef = np.concatenate([We, Wf], axis=1).astype(bf)
    shared = {
        "wq": (np.asarray(inputs["Wq"]) * scale).astype(bf),
        "wk": np.asarray(inputs["Wk"]).astype(bf),
        "wv": np.asarray(inputs["Wv"]).astype(bf),
        "wo": np.asarray(inputs["Wo"]).astype(bf),
        "wkaug": np.stack([np.asarray(inputs["bk"], f32), ones]).astype(bf),
        "auge": np.stack([sWe, np.asarray(inputs["be"], f32)]).astype(bf),
        "wvaug": np.stack([np.asarray(inputs["bv"], f32), ones]).astype(bf),
        "augf": np.stack([sWf, np.asarray(inputs["bf"], f32)]).astype(bf),
        "bq": (np.asarray(inputs["bq"]) * scale).astype(f32),
        "bo": np.asarray(inputs["bo"]).astype(f32),
    }
    in_maps = []
    for c in range(NCORES):
        b, half = c // 2, c % 2
        sl = slice(half * SH, (half + 1) * SH)
        m = dict(shared)
        m["qT"] = np.ascontiguousarray(q[b, sl, :].T).astype(bf)
        m["v"] = np.ascontiguousarray(v[b, sl, :]).astype(bf)
        m["wef"] = np.ascontiguousarray(wef[sl, :])
        in_maps.append(m)
    return in_maps


def kernel(**inputs):
    if "nc" not in _CACHE:
        _CACHE["nc"] = _build_kernel()
    nc = _CACHE["nc"]
    in_maps = _prep_inputs(inputs)
    res = bass_utils.run_bass_kernel_spmd(nc, in_maps, core_ids=list(range(NCORES)))
    out = np.empty((B, S, D), np.float32)
    for c in range(NCORES):
        b, half = c // 2, c % 2
        out[b, half * SH:(half + 1) * SH, :] = res.results[c]["out"].astype(np.float32)
    return out
